# revision 31
# baseline (speedup 1.0000x reference)
"""Density-aware Chamfer distance kernel for Trainium2 (Bass/Tile).

Contract: kernel(xyz1, xyz2) takes FULL inputs (8, 4096, 3) fp32 and
returns the FULL scalar output. The 8 point-cloud pairs are processed
PAIRS-per-core on B//PAIRS NeuronCores: the axon tunnel charges
~2.7ms per participating core on every dispatch (plus ~29ms base), so
fewer, busier cores beat 8 idle-ish ones for this tiny kernel.

Math note (avoids argmin indices / gathers entirely):
  loss_b = 1 - (S1 + S2) / (2N)  with
  S_d = sum_j T[j] * mask[j] / (c[j] + eps)
  c[j]  = #rows whose argmin is column j       (count)
  T[j]  = sum of exp(-1000*dmin_i) over rows i with argmin j
Both c and T are column sums of the one-hot argmin indicator
S[i,j] = [D[i,j] == rowmin_i], computed as 1 - Z with
Z = Sign(D - rowmin) in {0,1} and accumulated on the tensor engine
via Z^T @ [ones, exp] (complement form).

Host-side: the compiled executable (jax.jit of the bass_exec custom
call, shard_map over the used cores) is built once and cached; warm
calls only pay input prep + one dispatch. Inputs ship as ONE packed
[PAIRS*8, 4096] fp32 tensor per core (rows per pair: x | |x|^2 | y |
|y|^2); u/v matmul operands are assembled on-device via DMA
row-mapping, an in-place -2x scale, and iota/memset scratch rows.
"""

import numpy as np

B = 8
N = 4096
ALPHA = 1000.0
EPS = 1e-6

K = 6                # augmented contraction dim (xyz, norms, ones, tilt)
TILT = 2.0 ** -37    # tie-breaking tilt: D[i,j] += j*TILT (first-min wins)
P = 128              # rows per strip
NSTRIP = N // P      # 32 strips per direction
GROUP = 512          # D columns per PSUM group tile (1 bank)
NGROUP = N // GROUP  # 8
CHUNK = 512          # max fp32 matmul moving free dim
SUB = 128            # czT subchunk (matmul M limit)
RIN = 8              # packed input rows per pair: x | |x|^2 | y | |y|^2

PAIRS = 4            # point-cloud pairs per core
CORES = B // PAIRS   # cores participating in the dispatch
# Latency model (measured, drift-corrected): total ~ base (24-50ms,
# time-varying relay) + ~2.7ms/core + device time per pair; 2 cores
# x 4 pairs measured best (47.1ms) among validated configs.

_cache = {}
last_run_info = {}


def _build_nc(pairs=PAIRS, zbufs=2, sbufs=4, psbufs=8):
    import concourse.bacc as bacc
    import concourse.tile as tile
    from concourse import mybir

    f32 = mybir.dt.float32
    bf16 = mybir.dt.bfloat16
    X = mybir.AxisListType.X
    Alu = mybir.AluOpType
    Act = mybir.ActivationFunctionType

    nc = bacc.Bacc("TRN2", target_bir_lowering=False, debug=False)

    xin_dram = nc.declare_dram_parameter(
        "xin", [pairs * RIN, N], f32, isOutput=False)
    out_dram = nc.declare_dram_parameter("out", [1, 1], f32, isOutput=True)

    with tile.TileContext(nc) as tc:
        with (
            tc.tile_pool(name="uv", bufs=1) as uv_pool,
            tc.tile_pool(name="persist", bufs=1) as persist,
            tc.tile_pool(name="zbuf", bufs=zbufs) as zpool,
            tc.tile_pool(name="small", bufs=sbufs) as small,
            tc.tile_pool(name="ep", bufs=1) as ep,
            tc.tile_pool(name="ps", bufs=psbufs, space="PSUM") as psum,
        ):
            # shared scratch rows (engine ops need 32-aligned partition
            # bases, so rows landing at partitions 3-5 are placed with
            # DMA from these base-0 tiles)
            ones2 = persist.tile([2, N], f32, name="ones2")
            nc.vector.memset(ones2[:], 1.0)
            tiltrow = persist.tile([1, N], f32, name="tiltrow")
            nc.gpsimd.iota(tiltrow[:], pattern=[[1, N]], base=0,
                           channel_multiplier=0,
                           allow_small_or_imprecise_dtypes=True)
            nc.vector.tensor_scalar_mul(tiltrow[:], tiltrow[:], TILT)
            ones_sb = persist.tile([P, SUB], bf16, name="ones_sb")
            nc.vector.memset(ones_sb[:], 1.0)
            # running per-partition sum over pairs and directions
            sacc = persist.tile([P, 1], f32, name="sacc")
            nc.vector.memset(sacc[:], 0.0)

            ctw = 2 * (N // SUB) + 2

            for pp in range(pairs):
                o = pp * RIN  # row base of this pair in xin
                # assemble U/V operands (K=6 on partitions) from packed
                # rows: o+0:3=x o+3=|x|^2 o+4:7=y o+7=|y|^2
                u_sb = [None, None]
                v_sb = [None, None]
                for d in range(2):
                    u_sb[d] = uv_pool.tile(
                        [K, N], f32, name=f"u{d}sb", tag=f"u{d}")
                    v_sb[d] = uv_pool.tile(
                        [K, N], f32, name=f"v{d}sb", tag=f"v{d}")
                # u0 = (-2x | |x|^2 | 1 | 1)
                nc.sync.dma_start(out=u_sb[0][0:4, :],
                                  in_=xin_dram[o:o + 4, :])
                nc.vector.tensor_scalar_mul(
                    u_sb[0][0:3, :], u_sb[0][0:3, :], -2.0)
                nc.sync.dma_start(out=u_sb[0][4:6, :], in_=ones2[:])
                # v0 = (y | 1 | |y|^2 | j*TILT)
                nc.sync.dma_start(out=v_sb[0][0:3, :],
                                  in_=xin_dram[o + 4:o + 7, :])
                nc.sync.dma_start(out=v_sb[0][3:4, :], in_=ones2[0:1, :])
                nc.sync.dma_start(out=v_sb[0][4:5, :],
                                  in_=xin_dram[o + 7:o + 8, :])
                nc.sync.dma_start(out=v_sb[0][5:6, :], in_=tiltrow[:])
                # u1 = (-2y | |y|^2 | 1 | 1)
                nc.sync.dma_start(out=u_sb[1][0:4, :],
                                  in_=xin_dram[o + 4:o + 8, :])
                nc.vector.tensor_scalar_mul(
                    u_sb[1][0:3, :], u_sb[1][0:3, :], -2.0)
                nc.sync.dma_start(out=u_sb[1][4:6, :], in_=ones2[:])
                # v1 = (x | 1 | |x|^2 | j*TILT)
                nc.sync.dma_start(out=v_sb[1][0:3, :],
                                  in_=xin_dram[o:o + 3, :])
                nc.sync.dma_start(out=v_sb[1][3:4, :], in_=ones2[0:1, :])
                nc.sync.dma_start(out=v_sb[1][4:5, :],
                                  in_=xin_dram[o + 3:o + 4, :])
                nc.sync.dma_start(out=v_sb[1][5:6, :], in_=tiltrow[:])

                # per-direction accumulation slabs: per strip, 64 cols of
                # [cnt-complement, mass-complement] per j-subchunk + 2
                # cols [128, se_t] from the all-ones lhsT matmul
                cz_slab = [persist.tile([P, NSTRIP, ctw], f32,
                                        name=f"czslab{d}", tag=f"cz{d}")
                           for d in range(2)]  # [P, 32, 66]
                spart = [None, None]

                def emit_czt(d, t, zt, wt, cz_slab=cz_slab):
                    # count matmuls for strip t (deferred one iteration so
                    # PE never stalls on this strip's Sign)
                    ct = psum.tile([P, ctw], f32, name="ct", tag="dg")
                    for s in range(N // SUB):
                        nc.tensor.matmul(
                            ct[:, 2 * s:2 * s + 2],
                            lhsT=zt[:, s * SUB:(s + 1) * SUB],
                            rhs=wt[:],
                            start=True, stop=True,
                        )
                    # se_t with the same systolic accumulation tree as
                    # cz1_t, replicated to all partitions by the ones lhsT
                    nc.tensor.matmul(
                        ct[:, 2 * (N // SUB):ctw],
                        lhsT=ones_sb[:],
                        rhs=wt[:],
                        start=True, stop=True,
                    )
                    nc.vector.tensor_copy(cz_slab[d][:, t, :], ct[:])

                for d in range(2):
                    U, V = u_sb[d], v_sb[d]
                    pending = None
                    for t in range(NSTRIP):
                        lhsT = U[:, t * P:(t + 1) * P]
                        pm = small.tile([P, NGROUP], f32, name="pm",
                                        tag="pm")
                        zt = zpool.tile([P, N], bf16, name="zt", tag="z")
                        dgs = []
                        for g in range(NGROUP):
                            dg = psum.tile([P, GROUP], f32, name="dg",
                                           tag="dg")
                            dgs.append(dg)
                            for c in range(GROUP // CHUNK):
                                j0 = g * GROUP + c * CHUNK
                                nc.tensor.matmul(
                                    dg[:, c * CHUNK:(c + 1) * CHUNK],
                                    lhsT=lhsT,
                                    rhs=V[:, j0:j0 + CHUNK],
                                    start=True, stop=True,
                                )
                            nc.vector.tensor_reduce(
                                pm[:, g:g + 1], dg[:], axis=X, op=Alu.min)
                        rowmin = small.tile([P, 1], f32, name="rowmin",
                                            tag="rm")
                        nc.vector.tensor_reduce(
                            rowmin[:], pm[:], axis=X, op=Alu.min)
                        wt = small.tile([P, 2], bf16, name="wt", tag="w")
                        nc.vector.memset(wt[:, 0:1], 1.0)
                        nc.scalar.activation(
                            wt[:, 1:2], rowmin[:], Act.Exp, scale=-ALPHA)
                        for g in range(NGROUP):
                            # Z' = Sign(rowmin - D) in {0(min), -1(above)}
                            nc.scalar.activation(
                                zt[:, g * GROUP:(g + 1) * GROUP], dgs[g][:],
                                Act.Sign, bias=rowmin[:], scale=-1.0)
                        if pending is not None:
                            emit_czt(d, *pending)
                        pending = (t, zt, wt)
                    if pending is not None:
                        emit_czt(d, *pending)
                        pending = None

                    # ---- per-direction epilogue ----
                    nsub = N // SUB
                    # counts: c[j] = N - sum_t cz0_t[j] (exact int sums)
                    cz0 = cz_slab[d][:, :, 0:2 * nsub].rearrange(
                        "p t (s two) -> p s two t", two=2)[:, :, 0, :]
                    cz0sum = ep.tile([P, nsub], f32)
                    nc.vector.tensor_reduce(cz0sum[:], cz0, axis=X,
                                            op=Alu.add)
                    # per-strip row-sums of exp (PE-computed, same tree
                    # as cz1, already replicated across partitions)
                    se_row = cz_slab[d][:, :, ctw - 1]
                    # T[j] = sum_t (se_t - cz1_t[j])
                    tneg = ep.tile([P, nsub, NSTRIP], f32)
                    for s in range(nsub):
                        nc.vector.scalar_tensor_tensor(
                            out=tneg[:, s, :],
                            in0=cz_slab[d][:, :, 2 * s + 1],
                            scalar=1.0, in1=se_row,
                            op0=Alu.mult, op1=Alu.add)
                    tj = ep.tile([P, nsub], f32)
                    nc.vector.tensor_reduce(tj[:], tneg[:], axis=X,
                                            op=Alu.add)
                    c1 = ep.tile([P, nsub], f32)
                    nc.vector.tensor_scalar(
                        c1[:], cz0sum[:], 1.0, float(N),
                        op0=Alu.mult, op1=Alu.add)
                    c1e = ep.tile([P, nsub], f32)
                    nc.vector.tensor_scalar_add(c1e[:], c1[:], EPS)
                    r = ep.tile([P, nsub], f32)
                    nc.vector.reciprocal(r[:], c1e[:])
                    mask = ep.tile([P, nsub], f32)
                    nc.vector.tensor_scalar_min(mask[:], c1[:], 1.0)
                    rm = ep.tile([P, nsub], f32)
                    nc.vector.tensor_mul(rm[:], r[:], mask[:])
                    junk = ep.tile([P, nsub], f32)
                    sp = ep.tile([P, 1], f32, name=f"sp{d}", tag=f"sp{d}")
                    spart[d] = sp
                    nc.vector.tensor_mul(junk[:], tj[:], rm[:])
                    nc.vector.tensor_reduce(sp[:], junk[:], axis=X,
                                            op=Alu.add)

                sall = ep.tile([P, 1], f32, name="sall", tag="sall")
                nc.vector.tensor_add(sall[:], spart[0][:], spart[1][:])
                nc.vector.tensor_add(sacc[:], sacc[:], sall[:])

            stot = ep.tile([P, 1], f32)
            nc.gpsimd.partition_all_reduce(
                stot[:], sacc[:], channels=P, reduce_op=_reduce_op_add())
            nc.sync.dma_start(out=out_dram[:], in_=stot[0:1, 0:1])

    nc.compile()
    return nc


def _reduce_op_add():
    from concourse import bass_isa
    return bass_isa.ReduceOp.add


def _pack_inputs(xyz1, xyz2):
    """[B, RIN, N] packed rows per pair: x | |x|^2 | y | |y|^2."""
    xin = np.empty((B, RIN, N), np.float32)
    xin[:, 0:3] = xyz1.transpose(0, 2, 1)
    xin[:, 3] = np.einsum("bnd,bnd->bn", xyz1, xyz1)
    xin[:, 4:7] = xyz2.transpose(0, 2, 1)
    xin[:, 7] = np.einsum("bnd,bnd->bn", xyz2, xyz2)
    return xin


def _get_sharded(nc):
    """Build (once) the cached jit executable wrapping the bass_exec call."""
    import jax
    from jax.sharding import Mesh, PartitionSpec
    try:
        from jax.experimental.shard_map import shard_map
        _rep_kw = {"check_rep": False}
    except ImportError:
        from jax import shard_map
        _rep_kw = {"check_vma": False}
    from concourse import bass2jax, mybir

    bass2jax.install_neuronx_cc_hook()

    partition_name = (nc.partition_id_tensor.name
                      if nc.partition_id_tensor else None)
    in_names, out_names, out_avals = [], [], []
    for alloc in nc.m.functions[0].allocations:
        if not isinstance(alloc, mybir.MemoryLocationSet):
            continue
        name = alloc.memorylocations[0].name
        if alloc.kind == "ExternalInput":
            if name != partition_name:
                in_names.append(name)
        elif alloc.kind == "ExternalOutput":
            out_names.append(name)
            out_avals.append(jax.core.ShapedArray(
                tuple(alloc.tensor_shape), mybir.dt.np(alloc.dtype)))
    n_params = len(in_names)
    all_in_names = list(in_names) + list(out_names)
    if partition_name is not None:
        all_in_names.append(partition_name)
    donate = tuple(range(n_params, n_params + len(out_names)))

    def _body(*args):
        operands = list(args)
        if partition_name is not None:
            operands.append(bass2jax.partition_id_tensor())
        return tuple(bass2jax._bass_exec_p.bind(
            *operands,
            out_avals=tuple(out_avals),
            in_names=tuple(all_in_names),
            out_names=tuple(out_names),
            lowering_input_output_aliases=(),
            sim_require_finite=True,
            sim_require_nnan=True,
            nc=nc,
        ))

    devices = jax.devices()[:CORES]
    mesh = Mesh(np.asarray(devices), ("core",))
    nin = n_params + len(out_names)
    sharded = jax.jit(
        shard_map(_body, mesh=mesh,
                  in_specs=(PartitionSpec("core"),) * nin,
                  out_specs=(PartitionSpec("core"),) * len(out_names),
                  **_rep_kw),
        donate_argnums=donate, keep_unused=True,
    )
    return sharded, in_names, out_names, out_avals


def kernel(xyz1: np.ndarray, xyz2: np.ndarray) -> np.ndarray:
    xyz1 = np.asarray(xyz1, np.float32)
    xyz2 = np.asarray(xyz2, np.float32)
    assert xyz1.shape == (B, N, 3) and xyz2.shape == (B, N, 3)

    if "nc" not in _cache:
        _cache["nc"] = _build_nc(pairs=PAIRS)
    nc = _cache["nc"]

    xin = _pack_inputs(xyz1, xyz2)  # [B, RIN, N]

    if last_run_info.get("want_trace"):
        # profiling path: NTFF trace + true HW exec time (unavailable in
        # some containers — fall through to the fast path if it breaks)
        try:
            from concourse.bass_utils import run_bass_kernel_spmd
            in_maps = [
                {"xin": xin[c * PAIRS:(c + 1) * PAIRS].reshape(
                    PAIRS * RIN, N)}
                for c in range(CORES)]
            res = run_bass_kernel_spmd(
                nc, in_maps, core_ids=list(range(CORES)), trace=True)
            last_run_info["exec_time_ns"] = res.exec_time_ns
            last_run_info["profile_json"] = res.profile_json
            s = np.array([res.results[c]["out"][0, 0] for c in range(CORES)],
                         np.float64)
            return np.float32(1.0 - s.sum() / (B * 2 * N))
        except Exception as e:
            last_run_info["trace_error"] = repr(e)

    if "sharded" not in _cache:
        _cache["sharded"] = _get_sharded(nc)
    sharded, in_names, out_names, out_avals = _cache["sharded"]

    concat_in = [xin.reshape(CORES * PAIRS * RIN, N)]
    concat_zeros = [np.zeros((CORES * a.shape[0], *a.shape[1:]), a.dtype)
                    for a in out_avals]
    out_arrs = sharded(*concat_in, *concat_zeros)
    s = np.asarray(out_arrs[0], np.float64).reshape(CORES)
    return np.float32(1.0 - s.sum() / (B * 2 * N))
